# revision 60
# baseline (speedup 1.0000x reference)
"""Trainium2 Bass kernel for nn_EquivariantProductBasisBlock.

Math: per (node n, channel c), out[d] is a degree-3 polynomial in the
9-vector x[n,c,:] with coefficients depending on (element_type(n), c),
followed by a per-l channel-mixing linear layer and the sc skip-add.

Architecture (per core, 256 nodes, data-parallel over 8 cores):
  - monomials built in c-partition layout [128c, 64n, m] (m fastest)
  - ONE batched xbar DMA-transpose per (n-half, m-tile) to m-partition
    layout [m, 64n, 128c]  (f-low bits land on out partitions)
  - PE matmul with usym [m, 63] stationary; psum packs f-windows (t, t+8)
    on partition halves so drains use all 128 partitions
  - batched xbar back to [128c, 32nm, 128(h,j)]; wy multiply in-place
  - z matmul contracts channels with strided moving APs (no transposes)
  - PE transpose z, add sc, store.
"""
import numpy as np
from itertools import permutations

import bass_rust
import concourse.bass as bass
import concourse.bacc as bacc
import concourse.tile as tile
import concourse.mybir as mybir
from concourse.bass_utils import run_bass_kernel_spmd
from concourse.vector_clock import ScopedClock

# ---------------- problem constants (hardcoded per contest rules) ----------
N, C, E = 2048, 128, 10
NCORES = 8
NSH = N // NCORES            # 256 nodes per core
DIMS = [1, 3, 5]
P1, P2, P3 = 1, 2, 4
JL = [0, 7, 28]              # col base per l; col = JL[l] + q*(2l+1) + d
KB = [0, 128, 512]           # output col-block base per l
FP32 = mybir.dt.float32
BF16 = mybir.dt.bfloat16

# deg3 monomials (u<=v<=w) grouped by w, inner index tri(v)+u; 165 total.
# m-tile LO = deg3 rows 0..127.  HI tile (96 rows):
#   0:37 deg3 tail (m 128..164) | 37:82 deg2 | 82:91 deg1 (x_u) | 91:96 pad
H3, H2, H1 = 0, 37, 82


def tri(v):
    return v * (v + 1) // 2


OFF3 = [0]
for w in range(9):
    OFF3.append(OFF3[-1] + tri(w + 1))
assert OFF3[9] == 165


# ------------------------- host precompute --------------------------------

def build_usym(inputs):
    """usym [224, 64] float64: rows = (LO m | HI rows+128); col j' =
    JL[l] + q*(2l+1) + d  (q: 0-3 nu3, 4-5 nu2, 6 nu1)."""
    usym = np.zeros((256, 64), dtype=np.float64)

    def row3(u, v, w):
        m = OFF3[w] + tri(v) + u
        return m if m < 128 else 128 + H3 + (m - 128)

    for li in range(3):
        U1 = np.asarray(inputs[f"U1_{li}"], dtype=np.float64)
        U2 = np.asarray(inputs[f"U2_{li}"], dtype=np.float64)
        U3 = np.asarray(inputs[f"U3_{li}"], dtype=np.float64)
        dl = DIMS[li]
        for d in range(dl):
            for q in range(7):
                col = JL[li] + q * dl + d
                if q < 4:
                    T = U3[d, :, :, :, q]
                    for u in range(9):
                        for v in range(u, 9):
                            for w in range(v, 9):
                                s = sum(T[pm] for pm in set(permutations((u, v, w))))
                                usym[row3(u, v, w), col] = s
                elif q < 6:
                    T = U2[d, :, :, q - 4]
                    for u in range(9):
                        for v in range(u, 9):
                            s = sum(T[pm] for pm in set(permutations((u, v))))
                            usym[128 + H2 + tri(v) + u, col] = s
                else:
                    for u in range(9):
                        usym[128 + H1 + u, col] = U1[d, u, 0]
    return usym


def build_wallT(inputs):
    """wallT [16, 21*128] bf16-ready: wallT[e, (l*7+q)*128 + c] = W_nu[l][e,p,c]."""
    wallT = np.zeros((16, 21 * 128), dtype=np.float32)
    for li in range(3):
        Ws = [np.asarray(inputs["W3"][li]), np.asarray(inputs["W2"][li]),
              np.asarray(inputs["W1"][li])]
        for q in range(7):
            if q < 4:
                w = Ws[0][:, q, :]
            elif q < 6:
                w = Ws[1][:, q - 4, :]
            else:
                w = Ws[2][:, 0, :]
            wallT[0:E, (li * 7 + q) * 128:(li * 7 + q) * 128 + C] = w
    return wallT


def build_lwT(inputs):
    """lwT [128, 384]: lwT[c, l*128+f] = lin_w[l][c,f]/sqrt(C)."""
    lwT = np.zeros((128, 384), dtype=np.float32)
    isq = 1.0 / np.sqrt(np.float32(C))
    lw = np.asarray(inputs["lin_w"])
    for li in range(3):
        lwT[:, li * 128:(li + 1) * 128] = lw[li] * isq
    return lwT


# --------------------------- device program --------------------------------

class _TC(tile.TileContext):
    """TileContext with the final sync-engine drain split into 1-wait drains
    (this walrus build rejects >1 sem wait on a sync CTRL instruction)."""

    def _drain_and_barrier(self, tick_clock, wait_clock):
        drain_inst = self.nc.sync.drain()
        wait_clock.add_sem_waits(
            drain_inst.ins, ScopedClock({None: tick_clock.global_clock})
        )
        si = drain_inst.ins.sync_info
        waits = list(si.on_wait or []) if si else []
        if len(waits) > 1:
            si.on_wait = waits[:1]
            for w in waits[1:]:
                extra = self.nc.sync.drain()
                extra.ins.sync_info = bass_rust.SyncInfo(on_wait=[w], on_update=[])
        self.nc.all_engine_barrier()
        assert self.sems is not None
        popped = self.nc._tile_sem_poison_stack.pop()
        assert popped is self._sem_poison
        self.nc.clear_and_free_semaphores(list(self.sems.allocated().values()))
        self.nc.all_engine_barrier()


def _raw(ap_like, extra_offset, dims):
    """Build a raw AP on the same tensor: dims = [[step,count],...] incl. partition dim."""
    base = ap_like[:, :] if not isinstance(ap_like, bass.AP) else ap_like
    return bass.AP(tensor=base.tensor, offset=base.offset + extra_offset, ap=dims)


def build_program():
    nc = bacc.Bacc("TRN2", target_bir_lowering=False, debug=False)

    xtin = nc.dram_tensor("xtin", [128, 2304], BF16, kind="ExternalInput").ap()
    scin = nc.dram_tensor("scin", [NSH, 1152], BF16, kind="ExternalInput").ap()
    atin = nc.dram_tensor("atin", [16, 256], BF16, kind="ExternalInput").ap()
    usymA_d = nc.dram_tensor("usymA", [128, 64], BF16, kind="ExternalInput").ap()
    usymB_d = nc.dram_tensor("usymB", [128, 64], BF16, kind="ExternalInput").ap()
    wallT_d = nc.dram_tensor("wallT", [16, 2688], BF16, kind="ExternalInput").ap()
    lwT_d = nc.dram_tensor("lwT", [128, 384], BF16, kind="ExternalInput").ap()
    id16_d = nc.dram_tensor("id16", [128, 128], BF16, kind="ExternalInput").ap()
    yout = nc.dram_tensor("yout", [NSH, 1152], FP32, kind="ExternalOutput").ap()

    from contextlib import ExitStack
    with _TC(nc) as tc, ExitStack() as ctx:
        consts = ctx.enter_context(tc.tile_pool(name="consts", bufs=1))
        usymA = consts.tile([128, 64], BF16)
        usymB = consts.tile([128, 64], BF16)
        wallT = consts.tile([16, 2688], BF16)
        lwT = consts.tile([128, 384], BF16)
        id16 = consts.tile([128, 128], BF16)
        aT16 = consts.tile([16, 256], BF16)
        xT = consts.tile([128, 2, 9, 128], BF16)
        # pools
        io1_p = ctx.enter_context(tc.tile_pool(name="io1", bufs=1))
        mlo_p = ctx.enter_context(tc.tile_pool(name="mlo", bufs=2))
        mhi_p = ctx.enter_context(tc.tile_pool(name="mhi", bufs=2))
        mtlo_p = ctx.enter_context(tc.tile_pool(name="mtlo", bufs=2))
        mthi_p = ctx.enter_context(tc.tile_pool(name="mthi", bufs=2))
        g16_p = ctx.enter_context(tc.tile_pool(name="g16", bufs=2))
        gn_p = ctx.enter_context(tc.tile_pool(name="gn", bufs=4))
        z16_p = ctx.enter_context(tc.tile_pool(name="z16", bufs=1))
        wy_p = ctx.enter_context(tc.tile_pool(name="wy", bufs=1))
        ps_g = ctx.enter_context(tc.tile_pool(name="psg", bufs=2, space="PSUM"))
        ps_z = ctx.enter_context(tc.tile_pool(name="psz", bufs=1, space="PSUM"))
        ps_zt = ctx.enter_context(tc.tile_pool(name="pszt", bufs=1, space="PSUM"))
        ps_m = ctx.enter_context(tc.tile_pool(name="psm", bufs=1, space="PSUM"))

        # ---- prologue: host-swizzled xT / aT direct loads -----------------
        nc.sync.dma_start(
            out=xT, in_=xtin.rearrange("p (a b c) -> p a b c", b=9, c=128))
        nc.sync.dma_start(out=aT16, in_=atin)
        for t, d in [(usymA, usymA_d), (usymB, usymB_d), (wallT, wallT_d),
                     (lwT, lwT_d), (id16, id16_d)]:
            nc.sync.dma_start(out=t, in_=d)

        wyT16 = None

        def emit_wy(ch):
            nonlocal wyT16
            wyT16 = wy_p.tile([128, 21, 128], BF16, tag="wyT16")
            mv = aT16[0:10, ch * 128:(ch + 1) * 128]
            for b in range(6):
                nlq = 4 if b < 5 else 1
                wy_ps = ps_m.tile([128, 512], FP32, tag="psm_wy")
                for k in range(nlq):
                    lq = b * 4 + k
                    nc.tensor.matmul(wy_ps[:, k * 128:(k + 1) * 128],
                                     wallT[0:10, lq * 128:(lq + 1) * 128], mv,
                                     start=True, stop=True, skip_group_check=True)
                # drain -> wyT16[:, lq-range, ch-slice]
                o = _raw(wyT16[:, :, :], (b * 4) * 128,
                         [list(wyT16[:, :, :].ap[0]), [128, nlq], [1, 128]])
                nc.scalar.copy(o, wy_ps[:, 0:nlq * 128].rearrange(
                    "p (k n) -> p k n", n=128))

        gn2s = {}

        def emit_nh(ch, nh, last=False):
            xTc = xT[:, ch, :, :]
            nf = nh * 64
            gn = gn_p.tile([128, 32, 128], BF16, tag="gn")
            gn2s[(ch, nh)] = gn
            gnf = gn[:, :, :]

            # ---- mono build: HI tile then LO tile --------------------------
            mhi = mhi_p.tile([128, 64, 128], BF16, tag="mhi")
            nc.gpsimd.memset(mhi[:, :, 91:110], 0.0)
            nc.scalar.memzero(mhi[:, :, 110:128])
            # deg1: mhi[., n, H1+u] = xT[c, u, nf+n]
            xin_v = _raw(xTc, nf,
                         [list(xTc.ap[0]), [1, 64], [128, 9]])
            nc.scalar.copy(mhi[:, :, H1:H1 + 9], xin_v)
            xs = mhi[:, :, H1:H1 + 9]  # [c, 64n, 9] view of x
            # deg2: mhi[., n, H2+tri(v)+u] = x_u * x_v  (u<=v)
            for v in range(9):
                out = mhi[:, :, H2 + tri(v):H2 + tri(v) + v + 1]
                in0 = xs[:, :, 0:v + 1]
                in1 = xs[:, :, v:v + 1].broadcast_to((128, 64, v + 1))
                eng = nc.vector if v <= 4 else nc.gpsimd
                eng.tensor_mul(out, in0, in1)
            m2 = mhi[:, :, H2:H2 + 45]
            # deg3 HI (w=8 tail): mhi[., n, 0:37] = m2[8:45] * x_8
            nc.gpsimd.tensor_mul(
                mhi[:, :, 0:37], m2[:, :, 8:45],
                xs[:, :, 8:9].broadcast_to((128, 64, 37)))

            mlo = mlo_p.tile([128, 64, 128], BF16, tag="mlo")
            # deg3 LO by w: mlo[., n, OFF3[w]+t] = m2[t] * x_w
            for w in range(9):
                lo, hi = OFF3[w], min(OFF3[w + 1], 128)
                cnt = hi - lo
                eng = nc.gpsimd if w <= 4 else nc.vector
                eng.tensor_mul(
                    mlo[:, :, lo:hi], m2[:, :, 0:cnt],
                    xs[:, :, w:w + 1].broadcast_to((128, 64, cnt)))

            if nh == 0:
                emit_wy(ch)

            # ---- xbar mono -> m-partition layout: mhi on scalar queue so
            # the two transposes run concurrently (per-queue DMA parallelism)
            mthi = mthi_p.tile([128, 64, 128], BF16, tag="mthi")
            nc.sync.dma_start_transpose(
                mthi, mhi.rearrange("p a b -> p (a b)"))
            mtlo = mtlo_p.tile([128, 64, 128], BF16, tag="mtlo")
            nc.sync.dma_start_transpose(
                mtlo, mlo.rearrange("p a b -> p (a b)"))
            mtlo_f = mtlo.rearrange("p a b -> p (a b)")
            mthi_f = mthi.rearrange("p a b -> p (a b)")

            # ---- G matmul: psum packs f-windows (t, t+8) -------------------
            g16 = g16_p.tile([128, 4096], BF16, tag="g16")
            for t in range(8):
                g_ps = ps_g.tile([128, 512], FP32, tag="g_ps")
                for h in range(2):
                    sl = slice((t + 8 * h) * 512, (t + 8 * h + 1) * 512)
                    nc.tensor.matmul(g_ps[h * 64:h * 64 + 64, :],
                                     usymA, mtlo_f[:, sl],
                                     start=True, stop=False,
                                     skip_group_check=True)
                    nc.tensor.matmul(g_ps[h * 64:h * 64 + 64, :],
                                     usymB, mthi_f[:, sl],
                                     start=False, stop=True,
                                     skip_group_check=True)
                # drain (Act keeps DVE/Pool free for monomials)
                nc.scalar.copy(g16[:, t * 512:(t + 1) * 512], g_ps)

            # ---- xbar G -> gn [128c, 32nm, 128(h*64+j)] --------------------
            # last quarter: split so wy overlaps the 2nd transpose half
            if last:
                nc.sync.dma_start_transpose(gn[:, 0:16, :], g16[:, 0:2048])
                nc.sync.dma_start_transpose(gn[:, 16:32, :], g16[:, 2048:4096])
            else:
                nc.sync.dma_start_transpose(gn, g16)

            # ---- wy multiply (in place): gn *= wy --------------------------
            halves = ((0, 16), (16, 32)) if last else ((0, 32),)
            for b0, b1 in halves:
                for li in range(3):
                    dl = DIMS[li]
                    for h in range(2):
                        o = _raw(gnf, b0 * 128 + h * 64 + JL[li],
                                 [list(gnf.ap[0]), [128, b1 - b0],
                                  [dl, 7], [1, dl]])
                        i1 = _raw(wyT16[:, :, :],
                                  (li * 7) * 128 + nf + h * 32 + b0,
                                  [list(wyT16[:, :, :].ap[0]), [1, b1 - b0],
                                   [128, 7], [0, dl]])
                        eng = nc.vector if li == 2 else nc.gpsimd
                        eng.tensor_mul(o, o, i1)

        def emit_ztail(ch):
            n0 = ch * 128
            z16 = z16_p.tile([128, 9, 128], BF16, tag="z16")
            sc16 = io1_p.tile([128, 1152], BF16, tag="sc16")
            nc.sync.dma_start(out=sc16, in_=scin[n0:n0 + 128, :])

            # ---- z matmul (per n-half): contract c, accumulate over q ------
            for nh in range(2):
                nf = nh * 64
                gnf = gn2s[(ch, nh)][:, :, :]
                zA = ps_z.tile([128, 512], FP32, tag="zA")
                zB = ps_z.tile([128, 64], FP32, tag="zB")
                k = 0
                for li in range(3):
                    dl = DIMS[li]
                    for d in range(dl):
                        zout = zA[:, k * 64:(k + 1) * 64] if k < 8 else zB
                        for q in range(7):
                            mvg = _raw(gnf, JL[li] + q * dl + d,
                                       [list(gnf.ap[0]), [64, 2], [128, 32]])
                            nc.tensor.matmul(
                                zout, lwT[:, li * 128:(li + 1) * 128], mvg,
                                start=(q == 0), stop=(q == 6),
                                skip_group_check=True)
                        k += 1
                # drain z -> z16 [128f, 9ld, 128n]
                oA = _raw(z16[:, :, :], nf,
                          [list(z16[:, :, :].ap[0]), [128, 8], [1, 64]])
                nc.scalar.copy(oA, zA.rearrange("p (k n) -> p k n", n=64))
                nc.scalar.copy(z16[:, 8, nf:nf + 64], zB)

            # ---- zT (PE) + add sc + store ----------------------------------
            out32 = io1_p.tile([128, 1152], FP32, tag="out32")
            zt1 = ps_zt.tile([128, 4, 128], BF16, tag="zt1")
            zt2 = ps_zt.tile([128, 5, 128], BF16, tag="zt2")
            for kk in range(4):
                nc.tensor.transpose(zt1[:, kk, :], z16[:, kk, :], id16)
            for kk in range(5):
                nc.tensor.transpose(zt2[:, kk, :], z16[:, 4 + kk, :], id16)
            for li in range(3):
                dl = DIMS[li]
                o = _raw(out32[:, :], KB[li],
                         [list(out32[:, :].ap[0]), [dl, 128], [1, dl]])
                sv = _raw(sc16[:, :], KB[li],
                          [list(sc16[:, :].ap[0]), [dl, 128], [1, dl]])
                if li == 0:
                    zin = _raw(zt1[:, :, :], 0,
                               [list(zt1[:, :, :].ap[0]), [1, 128], [128, 1]])
                elif li == 1:
                    zin = _raw(zt1[:, :, :], 128,
                               [list(zt1[:, :, :].ap[0]), [1, 128], [128, 3]])
                else:
                    zin = _raw(zt2[:, :, :], 0,
                               [list(zt2[:, :, :].ap[0]), [1, 128], [128, 5]])
                nc.vector.tensor_add(o, zin, sv)
            nc.scalar.dma_start(out=yout[n0:n0 + 128, :], in_=out32)

        emit_nh(0, 0)
        emit_nh(0, 1)
        emit_nh(1, 0)
        emit_ztail(0)
        emit_nh(1, 1, last=True)
        emit_ztail(1)

    nc.compile()
    return nc


# --------------------------- public entry ---------------------------------

_PROG = None


def _get_prog():
    global _PROG
    if _PROG is None:
        _PROG = build_program()
    return _PROG


def host_constants(inputs):
    usym = build_usym(inputs)
    wallT = build_wallT(inputs)
    lwT = build_lwT(inputs)
    ident = np.eye(128, dtype=np.float32)
    import ml_dtypes
    return {
        "usymA": usym[0:128].astype(ml_dtypes.bfloat16),
        "usymB": usym[128:256].astype(ml_dtypes.bfloat16),
        "wallT": wallT.astype(ml_dtypes.bfloat16),
        "lwT": lwT.astype(ml_dtypes.bfloat16),
        "id16": ident.astype(ml_dtypes.bfloat16),
    }


def make_in_maps(inputs):
    import ml_dtypes
    consts = host_constants(inputs)
    nf = np.asarray(inputs["node_feats"], dtype=np.float32)      # [N,128,9]
    sc = np.asarray(inputs["sc"], dtype=np.float32)
    at = np.asarray(inputs["node_attrs"], dtype=np.float32)
    in_maps = []
    for cidx in range(NCORES):
        sl = slice(cidx * NSH, (cidx + 1) * NSH)
        # xT [128c, 2ch, 9w, 128n] -> [128, 2304]
        xc = nf[sl].reshape(2, 128, C, 9)                       # [2,128n,c,w]
        xt = np.transpose(xc, (2, 0, 3, 1)).reshape(128, 2304)  # [c,2,w,n]
        a16 = np.zeros((16, NSH), dtype=np.float32)
        a16[0:10] = at[sl].T
        m = {"xtin": np.ascontiguousarray(xt).astype(ml_dtypes.bfloat16),
             "scin": np.ascontiguousarray(sc[sl]).astype(ml_dtypes.bfloat16),
             "atin": np.ascontiguousarray(a16).astype(ml_dtypes.bfloat16)}
        m.update(consts)
        in_maps.append(m)
    return in_maps


def kernel(**inputs):
    nc = _get_prog()
    in_maps = make_in_maps(inputs)
    res = run_bass_kernel_spmd(nc, in_maps, list(range(NCORES)))
    out = np.concatenate([res.results[i]["yout"] for i in range(NCORES)], axis=0)
    return out.astype(np.float32)


# ----------------------- timing helpers (test.py only) ---------------------

def _build_runner(nc, in_maps):
    """Jitted 8-core sharded executor for an arbitrary bass program; returns
    (fn, device_args). No donation so the same device buffers can be reused."""
    import jax
    from jax.sharding import Mesh, PartitionSpec, NamedSharding
    from jax.experimental.shard_map import shard_map
    import concourse.mybir as mb
    from concourse import bass2jax

    bass2jax.install_neuronx_cc_hook()
    partition_name = nc.partition_id_tensor.name if nc.partition_id_tensor else None
    in_names, out_names, out_avals, zero_outs = [], [], [], []
    for alloc in nc.m.functions[0].allocations:
        if not isinstance(alloc, mb.MemoryLocationSet):
            continue
        name = alloc.memorylocations[0].name
        if alloc.kind == "ExternalInput":
            if name != partition_name:
                in_names.append(name)
        elif alloc.kind == "ExternalOutput":
            dt = mb.dt.np(alloc.dtype)
            out_avals.append(jax.core.ShapedArray(tuple(alloc.tensor_shape), dt))
            out_names.append(name)
            zero_outs.append(np.zeros(tuple(alloc.tensor_shape), dt))

    all_names = list(in_names) + list(out_names)
    if partition_name is not None:
        all_names.append(partition_name)

    def _body(*args):
        operands = list(args)
        if partition_name is not None:
            operands.append(bass2jax.partition_id_tensor())
        outs = bass2jax._bass_exec_p.bind(
            *operands,
            out_avals=tuple(out_avals),
            in_names=tuple(all_names),
            out_names=tuple(out_names),
            lowering_input_output_aliases=(),
            sim_require_finite=True,
            sim_require_nnan=True,
            nc=nc,
        )
        return tuple(outs)

    devices = jax.devices()[:NCORES]
    mesh = Mesh(np.asarray(devices), ("core",))
    nin = len(in_names) + len(zero_outs)
    fn = jax.jit(shard_map(_body, mesh=mesh,
                           in_specs=(PartitionSpec("core"),) * nin,
                           out_specs=(PartitionSpec("core"),) * len(out_names),
                           check_rep=False))
    sh = NamedSharding(mesh, PartitionSpec("core"))
    concat = [np.concatenate([m[n] for m in in_maps], axis=0) for n in in_names]
    concat += [np.concatenate([z] * NCORES, axis=0) for z in zero_outs]
    dargs = [jax.device_put(a, sh) for a in concat]
    return fn, dargs


def _build_trivial():
    """Minimal bass program for dispatch-overhead baseline."""
    nc = bacc.Bacc("TRN2", target_bir_lowering=False, debug=False)
    ti = nc.dram_tensor("tin", [128, 16], FP32, kind="ExternalInput").ap()
    to = nc.dram_tensor("tout", [128, 16], FP32, kind="ExternalOutput").ap()
    from contextlib import ExitStack
    with _TC(nc) as tc, ExitStack() as ctx:
        p = ctx.enter_context(tc.tile_pool(name="p", bufs=1))
        t = p.tile([128, 16], FP32)
        nc.sync.dma_start(out=t, in_=ti)
        nc.sync.dma_start(out=to, in_=t)
    nc.compile()
    return nc


def _time_fn(fn, dargs, iters):
    import time
    import jax
    o = fn(*dargs)
    jax.block_until_ready(o)
    best = float("inf")
    for _ in range(3):
        t0 = time.perf_counter()
        outs = [fn(*dargs) for _ in range(iters)]
        jax.block_until_ready(outs[-1])
        t1 = time.perf_counter()
        best = min(best, (t1 - t0) / iters)
    return best


def measure_hw_time(inputs, iters=32):
    nc = _get_prog()
    in_maps = make_in_maps(inputs)
    fn, dargs = _build_runner(nc, in_maps)
    t_full = _time_fn(fn, dargs, iters)

    tnc = _build_trivial()
    tmaps = [{"tin": np.zeros((128, 16), np.float32)} for _ in range(NCORES)]
    tfn, tdargs = _build_runner(tnc, tmaps)
    t_base = _time_fn(tfn, tdargs, iters)

    print(f"  per-call wall: full={t_full * 1e6:.1f}us base={t_base * 1e6:.1f}us")
    return max(t_full - t_base, 0.0) * 1e9


if __name__ == "__main__":
    nc = build_program()
    print("program built ok; instructions:",
          sum(len(b.instructions) for f in nc.m.functions for b in f.blocks))
